# revision 36
# baseline (speedup 1.0000x reference)
"""Trainium2 Bass kernel for nn_CGPODE (graph ODE message passing).

Math: reference computes NFE=8 Euler steps of dx/dt = A x over the node
dim (s_t = M^t x with M = I + 0.125*adj applied on the V axis), concats
the 9 states channel-wise, then applies a 1x1 conv (channel GEMM W) + b.

Refactoring used here (per batch n and lag l, x_nl is a [C,V] slab):
    out_nl = sum_t  W_t s_t           (W_t = W[:, t*C:(t+1)*C])
           = sum_t  M^t (W_t x_nl)    (channel mix commutes with node mix)
           = Horner: u_8 = z_8; u_t = z_t + M u_{t+1}; out_nl = u_0
with z_t = W_t-channel-mix of x computed straight from x via the PE
(x slab as the stationary operand), so every tensor stays node-on-partition
and no transposes or state materialization are needed.

Schedule: per batch n, two half-blocks of LH=6 lags. z for the next
half-block (PE matmuls + ACT psum->sbuf copies) overlaps the current
half-block's Horner sweep (PE matmuls + DVE adds); z is double-buffered.

v2 (ZFULL): z matmuls use FULL-ROW (K=128) stationaries — the host
pre-stacks each half-block's lag pair (l, l+3) onto the two partition
halves of x, and zero-padded wr variants (wr_top/wr_bot) select one lag
per stream.  One slab load then serves all four 288-col streams, and
every z weight-load background-buffers under in-flight full-row matmuls
exactly like the Horner's own loads (no row-group conflicts), which
removes all z<->Horner PE transition stalls.  Steady state: Horner
matmuls 162ns (384-col roofline), z matmuls ~123ns (288-col roofline),
PE busy ~91% of span.

Sharding: data-parallel over batch N across the 8 cores (adj/W replicated).
"""
import sys
if "/opt/trn_rl_repo" not in sys.path:
    sys.path.append("/opt/trn_rl_repo")  # fallback when axon_site paths absent
from contextlib import ExitStack

import numpy as np

import concourse.bacc as bacc
import concourse.tile as tile
from concourse import mybir
from concourse.bass_utils import run_bass_kernel_spmd

F32 = mybir.dt.float32
F16 = mybir.dt.float16
COPY = mybir.ActivationFunctionType.Copy

NFE = 8
STEP = 0.125
N, C, V, L = 64, 64, 500, 12
O = 64
T = NFE + 1          # 9 states
NCORES = 8
NPC = N // NCORES    # 8 batches per core
WT = 4               # node-dim tiles
VTILE = V // WT      # 125
LH = 6               # lags per half-block (cols per Horner matmul = LH*O = 384)
NHB = L // LH        # 2 half-blocks per batch
JT = T * O           # 576 z columns (t*O + o)
JH = JT // 2         # 288, half of the z columns per psum bank
import os
DT = F16
NPDT = np.float16
ZPAIR = True    # one multi-bank zps tile + one ACT copy per unit
UPAIR = False   # vt-paired horner psum drains measured slower; keep per-vt
ZFULL = True    # full-row z stationaries (lag pair stacked on K)


def build_nc(repeat=1):
    nc = bacc.Bacc(trn_type="TRN2", target_bir_lowering=False, debug=False)
    if ZFULL:
        # host pre-pairs lags: [n, hb, 0:C] = lags lo..lo+2, [C:2C] = lo+3..lo+5
        x_d = nc.dram_tensor("x", [NPC, NHB, 2 * C, V, LH // 2], DT,
                             kind="ExternalInput")
    else:
        x_d = nc.dram_tensor("x", [NPC, C, V, L], DT, kind="ExternalInput")
    mt_d = nc.dram_tensor("mt", [V, V], DT, kind="ExternalInput")
    wr_d = nc.dram_tensor("wr", [C, JT], DT, kind="ExternalInput")
    # [NPC, NHB, V, LH, O]: each per-(n,hb,vt) store is fully contiguous
    # (the [V, L, O] layout produced 128B bursts via the l-slice stride and
    # left ~12us of straggling DMA after the last compute op)
    out_d = nc.dram_tensor("out", [NPC, NHB, V, LH, O], DT,
                           kind="ExternalOutput")

    with tile.TileContext(nc) as tc, ExitStack() as ctx:
        rep = ctx.enter_context(tc.For_i(0, repeat, 1)) if repeat > 1 else None
        const = ctx.enter_context(tc.tile_pool(name="const", bufs=1))
        xp = ctx.enter_context(tc.tile_pool(name="xp", bufs=3))
        zp = ctx.enter_context(tc.tile_pool(name="zp", bufs=2))
        up = ctx.enter_context(tc.tile_pool(name="up", bufs=3))
        zps = ctx.enter_context(
            tc.tile_pool(name="zps", bufs=2 if ZFULL else (1 if ZPAIR else 2), space="PSUM"))
        hp = ctx.enter_context(
            tc.tile_pool(name="hp", bufs=2 if UPAIR else 4, space="PSUM"))

        hbs = [(n, hb) for n in range(NPC) for hb in range(NHB)]
        x_tiles = {}

        if ZFULL:
            # first z batch only needs x(0,0) + wr: issue that DMA first,
            # split 4-way across partition ranges so it spreads over DMA
            # queues (a single transfer runs ~3.6us; the prologue waits on it)
            x_tiles[(0, 0)] = xp.tile([2 * C, V, LH // 2], DT, tag="x",
                                      name="x_sb_0_0")
            for q in range(4):
                pr = slice(q * C // 2, (q + 1) * C // 2)
                nc.sync.dma_start(x_tiles[(0, 0)][pr], x_d.ap()[0, 0][pr])

        # zero-padded wr variants: full-row (K=128) z matmuls pick out
        # one lag of the stacked stationary via the zero half, so z
        # weight loads background-load under in-flight full-row matmuls
        # exactly like the horner's (no row-group conflicts).  Emitted
        # BEFORE the mt DMAs: the prologue z only needs x(0,0) + wr.
        wr_top = const.tile([2 * C, JT], DT, tag="wrt", name="wr_top")
        wr_bot = const.tile([2 * C, JT], DT, tag="wrb", name="wr_bot")
        nc.vector.memset(wr_top[C:2 * C, :], 0.0)
        nc.vector.memset(wr_bot[0:C, :], 0.0)
        nc.sync.dma_start(wr_top[0:C, :], wr_d.ap()[:])
        nc.sync.dma_start(wr_bot[C:2 * C, :], wr_d.ap()[:])

        # HAM warmup: the PE clock-gate starts at 1.2GHz and needs ~3.4us
        # of sustained matmul activity to release to 2.4GHz.  Burn dummy
        # matmuls on the (memset, DMA-independent) zero half of wr while
        # the x/wr DMAs are still in flight, so the real prologue z runs
        # at full clock.  A tiny psum->sbuf->dram sink keeps DCE away.
        warm_ps = hp.tile([VTILE, LH, O], F32, tag="hp", name="warm_ps")
        for i in range(14):
            nc.tensor.matmul(warm_ps[:], wr_top[C:2 * C, 0:VTILE],
                             wr_top[C:2 * C, 0:LH * O],
                             start=True, stop=True)
        def warm(k):
            for _ in range(k):
                nc.tensor.matmul(warm_ps[:], wr_top[C:2 * C, 0:VTILE],
                                 wr_top[C:2 * C, 0:LH * O],
                                 start=True, stop=True)

        # constants: M^T node-mix tiles
        mt_sb = []
        for wt in range(WT):
            t_ = const.tile([VTILE, V], DT, tag=f"mt{wt}", name=f"mt_sb{wt}")
            nc.sync.dma_start(t_[:], mt_d.ap()[wt * VTILE:(wt + 1) * VTILE, :])
            mt_sb.append(t_)

        z_tiles = {}

        def ensure_x(n, hb=None):
            # x duplicated onto partitions 64..127 so two lags can occupy
            # distinct PE row-groups (ZFULL: lag pair stacked on K instead:
            # top half holds lags lo..lo+2, bottom half lags lo+3..lo+5).
            if ZFULL:
                key = (n, hb)
                if key not in x_tiles:
                    x_tiles[key] = xp.tile([2 * C, V, LH // 2], DT, tag="x",
                                           name=f"x_sb_{n}_{hb}")
                    # 4-way split spreads the transfer over DMA queues
                    # (~1us instead of ~3.6us; first z units wait on it)
                    for q in range(4):
                        pr = slice(q * C // 2, (q + 1) * C // 2)
                        nc.sync.dma_start(x_tiles[key][pr],
                                          x_d.ap()[n, hb][pr])
                return x_tiles[key]
            if n not in x_tiles:
                x_tiles[n] = xp.tile([2 * C, V, L], DT, tag="x",
                                     name=f"x_sb_{n}")
                nc.sync.dma_start(x_tiles[n][0:C], x_d.ap()[n])
                nc.sync.dma_start(x_tiles[n][C:2 * C], x_d.ap()[n])
            return x_tiles[n]

        def make_z_units(j, prologue=False):
            """Closures that emit half-block j's z work:
            zbig[:, wt, li, t*O+o] = sum_c x[c, wt*125+w, lo+li] * W[o, t*C+c].
            Unit (lp, wt): lags (la, lb) = (lp, lp+3) of the half-block, four
            288-col matmuls (a-low, a-high, b-low, b-high), one ACT copy."""
            n, hb = hbs[j]
            x_sb = ensure_x(n, hb)
            zbig = zp.tile([VTILE, WT, LH, JT], DT, tag="z",
                           name=f"z_{n}_{hb}")
            z_tiles[j] = zbig
            units = []
            for lp in range(LH // 2):
                la, lb = hb * LH + lp, hb * LH + lp + LH // 2
                for wt in range(WT):
                    def unit(ch=0, borrow=True, lp=lp, la=la, lb=lb, wt=wt):
                        ws = slice(wt * VTILE, (wt + 1) * VTILE)
                        if ZFULL:
                            # one full-row stationary covers both lags; the
                            # four 288-col streams share it (weight reloads
                            # hide in the background buffer)
                            stat = x_sb[:, ws, lp]
                            if prologue:
                                # prologue subunit (one jt-chunk): horner
                                # psum banks are idle — borrow them for the
                                # b-half so two subunits pipeline, and split
                                # drains across engines
                                cs = slice(ch * JH, (ch + 1) * JH)
                                ps = zps.tile(
                                    [VTILE, 2, 512], F32, tag="zps",
                                    name=f"zps_{n}_{hb}_{lp}_{wt}_{ch}")
                                nc.tensor.matmul(
                                    ps[:, 0, 0:JH], stat, wr_top[:, cs],
                                    start=True, stop=True)
                                nc.scalar.activation(
                                    zbig[:, wt, lp, cs], ps[:, 0, 0:JH],
                                    COPY)
                                lagb = lp + LH // 2
                                if borrow:
                                    psb = hp.tile(
                                        [VTILE, LH, O], F32, tag="hp",
                                        name=f"zpb_{n}_{hb}_{lp}_{wt}_{ch}")
                                    pb = psb[:].rearrange(
                                        "p a b -> p (a b)")[:, 0:JH]
                                else:
                                    psb = zps.tile(
                                        [VTILE, 2, 512], F32, tag="zps",
                                        name=f"zpb_{n}_{hb}_{lp}_{wt}_{ch}")
                                    pb = psb[:, 0, 0:JH]
                                nc.tensor.matmul(pb, stat, wr_bot[:, cs],
                                                 start=True, stop=True)
                                nc.vector.tensor_copy(
                                    zbig[:, wt, lagb, cs], pb)
                            else:
                                for half, (wrv, lag) in enumerate(
                                        ((wr_top, lp), (wr_bot, lp + LH // 2))):
                                    ps = zps.tile(
                                        [VTILE, 2, 512], F32, tag="zps",
                                        name=f"zps_{n}_{hb}_{lp}_{wt}_{half}")
                                    for h in range(2):
                                        cs = slice(h * JH, (h + 1) * JH)
                                        nc.tensor.matmul(
                                            ps[:, h, 0:JH], stat, wrv[:, cs],
                                            start=True, stop=True)
                                    src = ps[:, :, 0:JH]
                                    dst = zbig[:, wt, lag, :].rearrange(
                                        "p (b d) -> p b d", b=2)
                                    nc.scalar.activation(dst, src, COPY)
                        elif ZPAIR:
                            ps = zps.tile([VTILE, 4, 512], F32, tag="zps",
                                          name=f"zps_{n}_{hb}_{lp}_{wt}")
                            for h in range(2):
                                cs = slice(h * JH, (h + 1) * JH)
                                nc.tensor.matmul(
                                    ps[:, h, 0:JH], x_sb[0:C, ws, la],
                                    wr_sb[0:C, cs], start=True, stop=True,
                                    tile_position=(0, 0))
                            for h in range(2):
                                cs = slice(h * JH, (h + 1) * JH)
                                nc.tensor.matmul(
                                    ps[:, 2 + h, 0:JH], x_sb[C:2 * C, ws, lb],
                                    wr_sb[C:2 * C, cs], start=True,
                                    stop=True, tile_position=(64, 0))
                            # one copy: [4, 288] psum chunks -> lags (lp, lp+3)
                            src = ps[:, :, 0:JH].rearrange(
                                "p (g b) d -> p g b d", g=2)
                            dst = zbig[:, wt, lp::LH // 2, :].rearrange(
                                "p g (b d) -> p g b d", b=2)
                            nc.scalar.activation(dst, src, COPY)
                        else:
                            psa = zps.tile([VTILE, 1024], F32, tag="zps",
                                           name=f"zpa_{n}_{hb}_{lp}_{wt}")
                            psb = zps.tile([VTILE, 1024], F32, tag="zps",
                                           name=f"zpb_{n}_{hb}_{lp}_{wt}")
                            for h in range(2):
                                cs = slice(h * JH, (h + 1) * JH)
                                nc.tensor.matmul(
                                    psa[:, h * 512:h * 512 + JH],
                                    x_sb[0:C, ws, la],
                                    wr_sb[0:C, cs], start=True, stop=True,
                                    tile_position=(0, 0))
                            for h in range(2):
                                cs = slice(h * JH, (h + 1) * JH)
                                nc.tensor.matmul(
                                    psb[:, h * 512:h * 512 + JH],
                                    x_sb[C:2 * C, ws, lb],
                                    wr_sb[C:2 * C, cs], start=True,
                                    stop=True, tile_position=(64, 0))
                            for li, pst in ((lp, psa), (lp + LH // 2, psb)):
                                src = pst[:].rearrange(
                                    "p (b d) -> p b d", b=2)[:, :, 0:JH]
                                dst = zbig[:, wt, li, :].rearrange(
                                    "p (b d) -> p b d", b=2)
                                nc.scalar.activation(dst, src, COPY)
                    units.append(unit)
            return units

        # prologue: z for half-block 0 emitted standalone.  The first few
        # units get dependency-free warm matmuls appended: they execute
        # while the next unit's weight-load waits on its psum-copy
        # semaphore, keeping the PE gapless so the HAM clock-gate
        # releases to 2.4GHz ~3.4us into the z phase instead of at the
        # first Horner sweep.
        _prologue_units = make_z_units(0, prologue=True)
        for unit in _prologue_units:
            unit(1)
        _c1_units = [(lambda u: (lambda: u(0, False)))(u)
                     for u in _prologue_units]
        warm_sb = const.tile([VTILE, 8], DT, tag="warmsb", name="warm_sb")
        nc.vector.tensor_copy(warm_sb[:], warm_ps[:, 0, 0:8])
        warm_d = nc.dram_tensor("warm_sink", [VTILE, 8], DT, kind="Internal")
        nc.sync.dma_start(warm_d.ap()[:], warm_sb[:])

        for k, (n, hb) in enumerate(hbs):
            if k + 2 < len(hbs):
                ensure_x(*hbs[k + 2])   # prefetch x one half-block earlier
            pending = make_z_units(k + 1) if k + 1 < len(hbs) else []
            if k == 0:
                # low-jt chunk of z(0) rides the first horner's pacing:
                # steps t=7..5 only touch jt>=288 (chunk 1), so chunk 0
                # lands just in time for t=4 at 4 pops per step
                pending = _c1_units + pending
            zbig = z_tiles[k]
            # Horner: u_8 = z_8 (read in place); u_t = z_t + M u_{t+1}
            u = None
            for t in range(NFE - 1, -1, -1):
                u_new = up.tile([VTILE, WT, LH, O], DT, tag="u",
                                name=f"u_{n}_{hb}_{t}")
                for vtp in range(WT // 2):
                    if UPAIR:
                        ps = hp.tile([VTILE, 2, 512], F32, tag="hp",
                                     name=f"hps_{n}_{hb}_{t}_{vtp}")
                        for jj in range(2):
                            vt = 2 * vtp + jj
                            lhs_col = slice(vt * VTILE, (vt + 1) * VTILE)
                            for wt in range(WT):
                                rhs = (zbig[:, wt, :, NFE * O:T * O]
                                       if t == NFE - 1 else u[:, wt])
                                nc.tensor.matmul(
                                    ps[:, jj, 0:LH * O],
                                    mt_sb[wt][:, lhs_col], rhs,
                                    start=(wt == 0), stop=(wt == WT - 1))
                        nc.vector.tensor_add(
                            u_new[:, 2 * vtp:2 * vtp + 2],
                            ps[:, :, 0:LH * O].rearrange(
                                "p g (a b) -> p g a b", a=LH),
                            zbig[:, 2 * vtp:2 * vtp + 2, :,
                                 t * O:(t + 1) * O])
                    else:
                        for jj in range(2):
                            vt = 2 * vtp + jj
                            lhs_col = slice(vt * VTILE, (vt + 1) * VTILE)
                            ps = hp.tile([VTILE, LH, O], F32, tag="hp",
                                         name=f"hps_{n}_{hb}_{t}_{vt}")
                            for wt in range(WT):
                                rhs = (zbig[:, wt, :, NFE * O:T * O]
                                       if t == NFE - 1 else u[:, wt])
                                nc.tensor.matmul(
                                    ps[:], mt_sb[wt][:, lhs_col], rhs,
                                    start=(wt == 0), stop=(wt == WT - 1))
                            nc.vector.tensor_add(
                                u_new[:, vt], ps[:],
                                zbig[:, vt, :, t * O:(t + 1) * O])
                            # next half-block's z fills PE step gaps, ONE
                            # unit per pop: ZFULL made z<->horner
                            # transitions free, and back-to-back units
                            # stall on each other's psum-copy release
                            # (2-buf zps rotation)
                            if pending and (k == 0 or jj == 1):
                                pending.pop(0)()
                u = u_new
            while pending:
                pending.pop(0)()
            del z_tiles[k]

            for vt in range(WT):
                vs = slice(vt * VTILE, (vt + 1) * VTILE)
                nc.sync.dma_start(out_d.ap()[n, hb, vs, :, :], u[:, vt])
    nc.compile()
    return nc


_NC_CACHE = None


def _get_nc(repeat=1):
    global _NC_CACHE
    if _NC_CACHE is None or _NC_CACHE[0] != repeat:
        _NC_CACHE = (repeat, build_nc(repeat))
    return _NC_CACHE[1]


def kernel(x, adj, W, b, _trace=False, _trace_kwargs=None, _repeat=1):
    x = np.ascontiguousarray(np.asarray(x, dtype=np.float32))
    adj = np.asarray(adj, dtype=np.float32)
    W = np.asarray(W, dtype=np.float32)
    b = np.asarray(b, dtype=np.float32)

    mt = np.ascontiguousarray((np.eye(V, dtype=np.float32) + STEP * adj).T.astype(NPDT))
    wr = np.ascontiguousarray(
        W.reshape(O, T, C).transpose(2, 1, 0).reshape(C, JT).astype(NPDT))
    x = x.astype(NPDT)
    if ZFULL:
        # [N, C, V, L] -> [N, NHB, 2C, V, LH//2]: lag pair (lp, lp+3) of each
        # half-block stacked on the partition dim
        xh = x.reshape(N, C, V, NHB, 2, LH // 2).transpose(0, 3, 4, 1, 2, 5)
        x = np.ascontiguousarray(xh.reshape(N, NHB, 2 * C, V, LH // 2))

    nc = _get_nc(_repeat)
    in_maps = [
        {"x": x[i * NPC:(i + 1) * NPC], "mt": mt, "wr": wr}
        for i in range(NCORES)
    ]
    kw = {}
    if _trace:
        kw["trace"] = True
        kw.update(_trace_kwargs or {})
    res = run_bass_kernel_spmd(nc, in_maps, list(range(NCORES)), **kw)
    out = np.concatenate([res.results[i]["out"] for i in range(NCORES)], axis=0)
    # [N, NHB, V, LH, O] -> [N, O, V, L]
    out = out.astype(np.float32).transpose(0, 4, 2, 1, 3).reshape(N, O, V, L)
    out = out + b[None, :, None, None]
    if _trace:
        return np.ascontiguousarray(out.astype(np.float32)), res
    return np.ascontiguousarray(out.astype(np.float32))


# revision 37
# speedup vs baseline: 1.0053x; 1.0053x over previous
"""Trainium2 Bass kernel for nn_CGPODE (graph ODE message passing).

Math: reference computes NFE=8 Euler steps of dx/dt = A x over the node
dim (s_t = M^t x with M = I + 0.125*adj applied on the V axis), concats
the 9 states channel-wise, then applies a 1x1 conv (channel GEMM W) + b.

Refactoring used here (per batch n and lag l, x_nl is a [C,V] slab):
    out_nl = sum_t  W_t s_t           (W_t = W[:, t*C:(t+1)*C])
           = sum_t  M^t (W_t x_nl)    (channel mix commutes with node mix)
           = Horner: u_8 = z_8; u_t = z_t + M u_{t+1}; out_nl = u_0
with z_t = W_t-channel-mix of x computed straight from x via the PE
(x slab as the stationary operand), so every tensor stays node-on-partition
and no transposes or state materialization are needed.

Schedule: per batch n, two half-blocks of LH=6 lags. z for the next
half-block (PE matmuls + ACT psum->sbuf copies) overlaps the current
half-block's Horner sweep (PE matmuls + DVE adds); z is double-buffered.

v2 (ZFULL): z matmuls use FULL-ROW (K=128) stationaries — the host
pre-stacks each half-block's lag pair (l, l+3) onto the two partition
halves of x, and zero-padded wr variants (wr_top/wr_bot) select one lag
per stream.  One slab load then serves all four 288-col streams, and
every z weight-load background-buffers under in-flight full-row matmuls
exactly like the Horner's own loads (no row-group conflicts), which
removes all z<->Horner PE transition stalls.  Steady state: Horner
matmuls 162ns (384-col roofline), z matmuls ~123ns (288-col roofline),
PE busy ~91% of span.

Sharding: data-parallel over batch N across the 8 cores (adj/W replicated).
"""
import sys
if "/opt/trn_rl_repo" not in sys.path:
    sys.path.append("/opt/trn_rl_repo")  # fallback when axon_site paths absent
from contextlib import ExitStack

import numpy as np

import concourse.bacc as bacc
import concourse.tile as tile
from concourse import mybir
from concourse.bass_utils import run_bass_kernel_spmd

F32 = mybir.dt.float32
F16 = mybir.dt.float16
COPY = mybir.ActivationFunctionType.Copy

NFE = 8
STEP = 0.125
N, C, V, L = 64, 64, 500, 12
O = 64
T = NFE + 1          # 9 states
NCORES = 8
NPC = N // NCORES    # 8 batches per core
WT = 4               # node-dim tiles
VTILE = V // WT      # 125
LH = 6               # lags per half-block (cols per Horner matmul = LH*O = 384)
NHB = L // LH        # 2 half-blocks per batch
JT = T * O           # 576 z columns (t*O + o)
JH = JT // 2         # 288, half of the z columns per psum bank
import os
DT = F16
NPDT = np.float16
ZPAIR = True    # one multi-bank zps tile + one ACT copy per unit
UPAIR = False   # vt-paired horner psum drains measured slower; keep per-vt
ZFULL = True    # full-row z stationaries (lag pair stacked on K)


def build_nc(repeat=1):
    nc = bacc.Bacc(trn_type="TRN2", target_bir_lowering=False, debug=False)
    if ZFULL:
        # host pre-pairs lags: [n, hb, 0:C] = lags lo..lo+2, [C:2C] = lo+3..lo+5
        x_d = nc.dram_tensor("x", [NPC, NHB, 2 * C, V, LH // 2], DT,
                             kind="ExternalInput")
    else:
        x_d = nc.dram_tensor("x", [NPC, C, V, L], DT, kind="ExternalInput")
    mt_d = nc.dram_tensor("mt", [V, V], DT, kind="ExternalInput")
    wr_d = nc.dram_tensor("wr", [C, JT], DT, kind="ExternalInput")
    # [NPC, NHB, V, LH, O]: each per-(n,hb,vt) store is fully contiguous
    # (the [V, L, O] layout produced 128B bursts via the l-slice stride and
    # left ~12us of straggling DMA after the last compute op)
    out_d = nc.dram_tensor("out", [NPC, NHB, V, LH, O], DT,
                           kind="ExternalOutput")

    with tile.TileContext(nc) as tc, ExitStack() as ctx:
        rep = ctx.enter_context(tc.For_i(0, repeat, 1)) if repeat > 1 else None
        const = ctx.enter_context(tc.tile_pool(name="const", bufs=1))
        xp = ctx.enter_context(tc.tile_pool(name="xp", bufs=3))
        zp = ctx.enter_context(tc.tile_pool(name="zp", bufs=2))
        up = ctx.enter_context(tc.tile_pool(name="up", bufs=3))
        zps = ctx.enter_context(
            tc.tile_pool(name="zps", bufs=2 if ZFULL else (1 if ZPAIR else 2), space="PSUM"))
        hp = ctx.enter_context(
            tc.tile_pool(name="hp", bufs=2 if UPAIR else 4, space="PSUM"))

        hbs = [(n, hb) for n in range(NPC) for hb in range(NHB)]
        x_tiles = {}

        if ZFULL:
            # first z batch only needs x(0,0) + wr: issue that DMA first,
            # split 4-way across partition ranges so it spreads over DMA
            # queues (a single transfer runs ~3.6us; the prologue waits on it)
            x_tiles[(0, 0)] = xp.tile([2 * C, V, LH // 2], DT, tag="x",
                                      name="x_sb_0_0")
            for q in range(4):
                pr = slice(q * C // 2, (q + 1) * C // 2)
                nc.sync.dma_start(x_tiles[(0, 0)][pr], x_d.ap()[0, 0][pr])

        # zero-padded wr variants: full-row (K=128) z matmuls pick out
        # one lag of the stacked stationary via the zero half, so z
        # weight loads background-load under in-flight full-row matmuls
        # exactly like the horner's (no row-group conflicts).  Emitted
        # BEFORE the mt DMAs: the prologue z only needs x(0,0) + wr.
        wr_top = const.tile([2 * C, JT], DT, tag="wrt", name="wr_top")
        wr_bot = const.tile([2 * C, JT], DT, tag="wrb", name="wr_bot")
        nc.vector.memset(wr_top[C:2 * C, :], 0.0)
        nc.vector.memset(wr_bot[0:C, :], 0.0)
        nc.sync.dma_start(wr_top[0:C, :], wr_d.ap()[:])
        nc.sync.dma_start(wr_bot[C:2 * C, :], wr_d.ap()[:])

        # HAM warmup: the PE clock-gate starts at 1.2GHz and needs ~3.4us
        # of sustained matmul activity to release to 2.4GHz.  Burn dummy
        # matmuls on the (memset, DMA-independent) zero half of wr while
        # the x/wr DMAs are still in flight, so the real prologue z runs
        # at full clock.  A tiny psum->sbuf->dram sink keeps DCE away.
        warm_ps = hp.tile([VTILE, LH, O], F32, tag="hp", name="warm_ps")
        for i in range(14):
            nc.tensor.matmul(warm_ps[:], wr_top[C:2 * C, 0:VTILE],
                             wr_top[C:2 * C, 0:LH * O],
                             start=True, stop=True)
        def warm(k):
            for _ in range(k):
                nc.tensor.matmul(warm_ps[:], wr_top[C:2 * C, 0:VTILE],
                                 wr_top[C:2 * C, 0:LH * O],
                                 start=True, stop=True)

        # constants: M^T node-mix tiles
        mt_sb = []
        for wt in range(WT):
            t_ = const.tile([VTILE, V], DT, tag=f"mt{wt}", name=f"mt_sb{wt}")
            nc.sync.dma_start(t_[:], mt_d.ap()[wt * VTILE:(wt + 1) * VTILE, :])
            mt_sb.append(t_)

        z_tiles = {}

        def ensure_x(n, hb=None):
            # x duplicated onto partitions 64..127 so two lags can occupy
            # distinct PE row-groups (ZFULL: lag pair stacked on K instead:
            # top half holds lags lo..lo+2, bottom half lags lo+3..lo+5).
            if ZFULL:
                key = (n, hb)
                if key not in x_tiles:
                    x_tiles[key] = xp.tile([2 * C, V, LH // 2], DT, tag="x",
                                           name=f"x_sb_{n}_{hb}")
                    # 4-way split spreads the transfer over DMA queues
                    # (~1us instead of ~3.6us; first z units wait on it)
                    for q in range(4):
                        pr = slice(q * C // 2, (q + 1) * C // 2)
                        nc.sync.dma_start(x_tiles[key][pr],
                                          x_d.ap()[n, hb][pr])
                return x_tiles[key]
            if n not in x_tiles:
                x_tiles[n] = xp.tile([2 * C, V, L], DT, tag="x",
                                     name=f"x_sb_{n}")
                nc.sync.dma_start(x_tiles[n][0:C], x_d.ap()[n])
                nc.sync.dma_start(x_tiles[n][C:2 * C], x_d.ap()[n])
            return x_tiles[n]

        def make_z_units(j, prologue=False):
            """Closures that emit half-block j's z work:
            zbig[:, wt, li, t*O+o] = sum_c x[c, wt*125+w, lo+li] * W[o, t*C+c].
            Unit (lp, wt): lags (la, lb) = (lp, lp+3) of the half-block, four
            288-col matmuls (a-low, a-high, b-low, b-high), one ACT copy."""
            n, hb = hbs[j]
            x_sb = ensure_x(n, hb)
            zbig = zp.tile([VTILE, WT, LH, JT], DT, tag="z",
                           name=f"z_{n}_{hb}")
            z_tiles[j] = zbig
            units = []
            for lp in range(LH // 2):
                la, lb = hb * LH + lp, hb * LH + lp + LH // 2
                for wt in range(WT):
                    def unit(ch=0, borrow=True, lp=lp, la=la, lb=lb, wt=wt):
                        ws = slice(wt * VTILE, (wt + 1) * VTILE)
                        if ZFULL:
                            # one full-row stationary covers both lags; the
                            # four 288-col streams share it (weight reloads
                            # hide in the background buffer)
                            stat = x_sb[:, ws, lp]
                            if prologue:
                                # prologue subunit (one jt-chunk): horner
                                # psum banks are idle — borrow them for the
                                # b-half so two subunits pipeline, and split
                                # drains across engines
                                cs = slice(ch * JH, (ch + 1) * JH)
                                ps = zps.tile(
                                    [VTILE, 2, 512], F32, tag="zps",
                                    name=f"zps_{n}_{hb}_{lp}_{wt}_{ch}")
                                nc.tensor.matmul(
                                    ps[:, 0, 0:JH], stat, wr_top[:, cs],
                                    start=True, stop=True)
                                nc.scalar.activation(
                                    zbig[:, wt, lp, cs], ps[:, 0, 0:JH],
                                    COPY)
                                lagb = lp + LH // 2
                                if borrow:
                                    psb = hp.tile(
                                        [VTILE, LH, O], F32, tag="hp",
                                        name=f"zpb_{n}_{hb}_{lp}_{wt}_{ch}")
                                    pb = psb[:].rearrange(
                                        "p a b -> p (a b)")[:, 0:JH]
                                else:
                                    psb = zps.tile(
                                        [VTILE, 2, 512], F32, tag="zps",
                                        name=f"zpb_{n}_{hb}_{lp}_{wt}_{ch}")
                                    pb = psb[:, 0, 0:JH]
                                nc.tensor.matmul(pb, stat, wr_bot[:, cs],
                                                 start=True, stop=True)
                                nc.vector.tensor_copy(
                                    zbig[:, wt, lagb, cs], pb)
                            else:
                                for half, (wrv, lag) in enumerate(
                                        ((wr_top, lp), (wr_bot, lp + LH // 2))):
                                    ps = zps.tile(
                                        [VTILE, 2, 512], F32, tag="zps",
                                        name=f"zps_{n}_{hb}_{lp}_{wt}_{half}")
                                    for h in range(2):
                                        cs = slice(h * JH, (h + 1) * JH)
                                        nc.tensor.matmul(
                                            ps[:, h, 0:JH], stat, wrv[:, cs],
                                            start=True, stop=True)
                                    src = ps[:, :, 0:JH]
                                    dst = zbig[:, wt, lag, :].rearrange(
                                        "p (b d) -> p b d", b=2)
                                    nc.scalar.activation(dst, src, COPY)
                        elif ZPAIR:
                            ps = zps.tile([VTILE, 4, 512], F32, tag="zps",
                                          name=f"zps_{n}_{hb}_{lp}_{wt}")
                            for h in range(2):
                                cs = slice(h * JH, (h + 1) * JH)
                                nc.tensor.matmul(
                                    ps[:, h, 0:JH], x_sb[0:C, ws, la],
                                    wr_sb[0:C, cs], start=True, stop=True,
                                    tile_position=(0, 0))
                            for h in range(2):
                                cs = slice(h * JH, (h + 1) * JH)
                                nc.tensor.matmul(
                                    ps[:, 2 + h, 0:JH], x_sb[C:2 * C, ws, lb],
                                    wr_sb[C:2 * C, cs], start=True,
                                    stop=True, tile_position=(64, 0))
                            # one copy: [4, 288] psum chunks -> lags (lp, lp+3)
                            src = ps[:, :, 0:JH].rearrange(
                                "p (g b) d -> p g b d", g=2)
                            dst = zbig[:, wt, lp::LH // 2, :].rearrange(
                                "p g (b d) -> p g b d", b=2)
                            nc.scalar.activation(dst, src, COPY)
                        else:
                            psa = zps.tile([VTILE, 1024], F32, tag="zps",
                                           name=f"zpa_{n}_{hb}_{lp}_{wt}")
                            psb = zps.tile([VTILE, 1024], F32, tag="zps",
                                           name=f"zpb_{n}_{hb}_{lp}_{wt}")
                            for h in range(2):
                                cs = slice(h * JH, (h + 1) * JH)
                                nc.tensor.matmul(
                                    psa[:, h * 512:h * 512 + JH],
                                    x_sb[0:C, ws, la],
                                    wr_sb[0:C, cs], start=True, stop=True,
                                    tile_position=(0, 0))
                            for h in range(2):
                                cs = slice(h * JH, (h + 1) * JH)
                                nc.tensor.matmul(
                                    psb[:, h * 512:h * 512 + JH],
                                    x_sb[C:2 * C, ws, lb],
                                    wr_sb[C:2 * C, cs], start=True,
                                    stop=True, tile_position=(64, 0))
                            for li, pst in ((lp, psa), (lp + LH // 2, psb)):
                                src = pst[:].rearrange(
                                    "p (b d) -> p b d", b=2)[:, :, 0:JH]
                                dst = zbig[:, wt, li, :].rearrange(
                                    "p (b d) -> p b d", b=2)
                                nc.scalar.activation(dst, src, COPY)
                    units.append(unit)
            return units

        # prologue: z for half-block 0 emitted standalone.  The first few
        # units get dependency-free warm matmuls appended: they execute
        # while the next unit's weight-load waits on its psum-copy
        # semaphore, keeping the PE gapless so the HAM clock-gate
        # releases to 2.4GHz ~3.4us into the z phase instead of at the
        # first Horner sweep.
        _prologue_units = make_z_units(0, prologue=True)
        for unit in _prologue_units:
            unit(1)
        _c1_units = [(lambda u: (lambda: u(0, False)))(u)
                     for u in _prologue_units]
        warm_sb = const.tile([VTILE, 8], DT, tag="warmsb", name="warm_sb")
        nc.vector.tensor_copy(warm_sb[:], warm_ps[:, 0, 0:8])
        warm_d = nc.dram_tensor("warm_sink", [VTILE, 8], DT, kind="Internal")
        nc.sync.dma_start(warm_d.ap()[:], warm_sb[:])

        for k, (n, hb) in enumerate(hbs):
            if k + 2 < len(hbs):
                ensure_x(*hbs[k + 2])   # prefetch x one half-block earlier
            pending = make_z_units(k + 1) if k + 1 < len(hbs) else []
            if k == 0:
                # low-jt chunk of z(0) rides the first horner's pacing:
                # steps t=7..5 only touch jt>=288 (chunk 1), so chunk 0
                # lands just in time for t=4 at 4 pops per step
                pending = _c1_units + pending
            zbig = z_tiles[k]
            # Horner: u_8 = z_8 (read in place); u_t = z_t + M u_{t+1}
            u = None
            for t in range(NFE - 1, -1, -1):
                u_new = up.tile([VTILE, WT, LH, O], DT, tag="u",
                                name=f"u_{n}_{hb}_{t}")
                for vtp in range(WT // 2):
                    if UPAIR:
                        ps = hp.tile([VTILE, 2, 512], F32, tag="hp",
                                     name=f"hps_{n}_{hb}_{t}_{vtp}")
                        for jj in range(2):
                            vt = 2 * vtp + jj
                            lhs_col = slice(vt * VTILE, (vt + 1) * VTILE)
                            for wt in range(WT):
                                rhs = (zbig[:, wt, :, NFE * O:T * O]
                                       if t == NFE - 1 else u[:, wt])
                                nc.tensor.matmul(
                                    ps[:, jj, 0:LH * O],
                                    mt_sb[wt][:, lhs_col], rhs,
                                    start=(wt == 0), stop=(wt == WT - 1))
                        nc.vector.tensor_add(
                            u_new[:, 2 * vtp:2 * vtp + 2],
                            ps[:, :, 0:LH * O].rearrange(
                                "p g (a b) -> p g a b", a=LH),
                            zbig[:, 2 * vtp:2 * vtp + 2, :,
                                 t * O:(t + 1) * O])
                    else:
                        for jj in range(2):
                            vt = 2 * vtp + jj
                            lhs_col = slice(vt * VTILE, (vt + 1) * VTILE)
                            ps = hp.tile([VTILE, LH, O], F32, tag="hp",
                                         name=f"hps_{n}_{hb}_{t}_{vt}")
                            for wt in range(WT):
                                rhs = (zbig[:, wt, :, NFE * O:T * O]
                                       if t == NFE - 1 else u[:, wt])
                                nc.tensor.matmul(
                                    ps[:], mt_sb[wt][:, lhs_col], rhs,
                                    start=(wt == 0), stop=(wt == WT - 1))
                            nc.vector.tensor_add(
                                u_new[:, vt], ps[:],
                                zbig[:, vt, :, t * O:(t + 1) * O])
                    # next half-block's z fills PE step gaps, one unit per
                    # half-step: ZFULL made z<->horner transitions free,
                    # and back-to-back units stall on each other's
                    # psum-copy release (2-buf zps rotation)
                    for _ in range(2 if k == 0 else 1):
                        if pending:
                            pending.pop(0)()
                u = u_new
            while pending:
                pending.pop(0)()
            del z_tiles[k]

            for vt in range(WT):
                vs = slice(vt * VTILE, (vt + 1) * VTILE)
                nc.sync.dma_start(out_d.ap()[n, hb, vs, :, :], u[:, vt])
    nc.compile()
    return nc


_NC_CACHE = None


def _get_nc(repeat=1):
    global _NC_CACHE
    if _NC_CACHE is None or _NC_CACHE[0] != repeat:
        _NC_CACHE = (repeat, build_nc(repeat))
    return _NC_CACHE[1]


def kernel(x, adj, W, b, _trace=False, _trace_kwargs=None, _repeat=1):
    x = np.ascontiguousarray(np.asarray(x, dtype=np.float32))
    adj = np.asarray(adj, dtype=np.float32)
    W = np.asarray(W, dtype=np.float32)
    b = np.asarray(b, dtype=np.float32)

    mt = np.ascontiguousarray((np.eye(V, dtype=np.float32) + STEP * adj).T.astype(NPDT))
    wr = np.ascontiguousarray(
        W.reshape(O, T, C).transpose(2, 1, 0).reshape(C, JT).astype(NPDT))
    x = x.astype(NPDT)
    if ZFULL:
        # [N, C, V, L] -> [N, NHB, 2C, V, LH//2]: lag pair (lp, lp+3) of each
        # half-block stacked on the partition dim
        xh = x.reshape(N, C, V, NHB, 2, LH // 2).transpose(0, 3, 4, 1, 2, 5)
        x = np.ascontiguousarray(xh.reshape(N, NHB, 2 * C, V, LH // 2))

    nc = _get_nc(_repeat)
    in_maps = [
        {"x": x[i * NPC:(i + 1) * NPC], "mt": mt, "wr": wr}
        for i in range(NCORES)
    ]
    kw = {}
    if _trace:
        kw["trace"] = True
        kw.update(_trace_kwargs or {})
    res = run_bass_kernel_spmd(nc, in_maps, list(range(NCORES)), **kw)
    out = np.concatenate([res.results[i]["out"] for i in range(NCORES)], axis=0)
    # [N, NHB, V, LH, O] -> [N, O, V, L]
    out = out.astype(np.float32).transpose(0, 4, 2, 1, 3).reshape(N, O, V, L)
    out = out + b[None, :, None, None]
    if _trace:
        return np.ascontiguousarray(out.astype(np.float32)), res
    return np.ascontiguousarray(out.astype(np.float32))
